# revision 17
# baseline (speedup 1.0000x reference)
"""Cross-attention kernel for Trainium2 (8 NeuronCores, SPMD).

Problem: B=4, Nq=1024, Nk=2048, D=512, 8 heads x 64 head-dim, fp32,
full-tensor bias added to scores before softmax.

Sharding: (batch, query-half) -> 8 disjoint shards, one per core. Each core
computes its own (512, 512) slice of the output; no collectives needed.
K/V projections are computed redundantly on the two cores sharing a batch.

Device layout: attention tensors kept transposed (feature/key dim on
partitions) so every matmul contraction lands on the partition axis:
  QT[d, q] = (SCALE*Wq) @ xT          KT[d, k] = Wk @ ctxT
  V[k, i]  = ctxT.T @ Wv.T
  ST[k, q] = KT_h.T @ QT_h            (two heads of a pair run concurrently
                                       in PE row groups 0-1 / 2-3)
  E = exp(ST) * exp(biasT - 4)        (ACT exp -> elementwise multiply;
                                       host sends exp(bias - 4).T so the
                                       bias add becomes a multiply; the -4
                                       cancels in the normalization)
  out2T[i(+1), q] = [V_h | 1].T @ E   (ones column yields softmax row-sums
                                       in the same accumulation)
  OT = out2T[0:64] * recip(sum)
  yT[d, q] = Wo @ OT + bo

Scheduling (the ACT exp chain, 64 x [128,1024] tiles ~ 71us, is the
pacer; everything else hides under it):
 - scores run TWO chunks ahead of the AV matmuls on the in-order PE
   queue, so neither the exp nor the eB multiply latency gates the PE;
 - eB is stored duplicated ([128, 2, 512] per chunk, dup'd by the DMA)
   so the multiply is one fully-contiguous fp16 op in the DVE 2x mode;
   every 3rd multiply runs on the otherwise-idle GPSIMD engine;
 - K/Q/V projections for later head pairs ride as PE gap filler;
 - normalization: row sums reshape to [128, 4] by DMA for a cheap
   4-elem/lane reciprocal, DRAM-bounce partition broadcast, lazy and off
   the critical path; the LAST pair instead uses an on-chip path (1-lane
   reciprocal + PE rank-1 broadcast) to avoid tail DMA latency;
 - the output projection runs entirely in the tail with all 8 PSUM banks
   (ki-outer): the ki<3 partials overlap the last pair's normalization;
 - host packs every input into SBUF-tile layout so each tensor loads with
   few large contiguous DMAs, ordered so the pair-0 K/Q projections
   start after ~1MB of DMA; eB streams per-chunk behind them.
Matmul operands are fp16 (fp32 PSUM accumulate).
"""

import numpy as np
import concourse.bass as bass
import concourse.bacc as bacc
import concourse.mybir as mybir
import concourse.tile as tile
from concourse import bass_utils

HEADS = 8
DH = 64
D = 512
NQ = 512          # queries per core (Nq=1024 split in halves)
NK = 2048
KC = NK // 128    # 16 key chunks
SCALE = DH ** -0.5
BSHIFT = 4.0      # exp(bias - BSHIFT): keeps fp16 weights in range

F32 = mybir.dt.float32
F16 = mybir.dt.float16
AF = mybir.ActivationFunctionType


def _bcast2(ap, n):
    """[128, F] -> [128, n, F] with a step-0 middle dim."""
    return bass.AP(ap.tensor, ap.offset, [ap.ap[0], [0, n], ap.ap[1]])


def _build_nc():
    nc = bacc.Bacc("TRN2", target_bir_lowering=False, debug=False)

    # All inputs host-packed into SBUF-tile layout [128, free].
    x_d = nc.dram_tensor("xp", [128, 4 * NQ], F16, kind="ExternalInput")
    ct_d = nc.dram_tensor("ctp", [128, 16 * NQ], F16, kind="ExternalInput")
    eb_d = nc.dram_tensor("ebp", [128, KC * 2 * NQ], F16, kind="ExternalInput")
    wq_d = nc.dram_tensor("wqp", [128, 2048], F16, kind="ExternalInput")
    wk_d = nc.dram_tensor("wkp", [128, 2048], F16, kind="ExternalInput")
    wv_d = nc.dram_tensor("wvp", [128, 2048], F16, kind="ExternalInput")
    wo_d = nc.dram_tensor("wop", [128, 2048], F16, kind="ExternalInput")
    bo_d = nc.dram_tensor("bop", [128, 4], F32, kind="ExternalInput")
    yT_d = nc.dram_tensor("yT", [D, NQ], F32, kind="ExternalOutput")
    scr_d = [nc.dram_tensor(f"scr{h}", [NQ], F16) for h in range(HEADS)]

    with tile.TileContext(nc) as tc, nc.allow_low_precision(
            reason="fp16 matmul operands, fp32 accumulation"):
        with (
            tc.tile_pool(name="const", bufs=1) as const,
            tc.tile_pool(name="main", bufs=1) as main,
            tc.tile_pool(name="work", bufs=8) as work,
            tc.tile_pool(name="norm", bufs=3) as norm,
        ):
            # ---- persistent SBUF tiles ----
            WK = const.tile([128, 2048], F16, name="WK", tag="WK")
            WQ = const.tile([128, 2048], F16, name="WQ", tag="WQ")
            WV = const.tile([128, 2048], F16, name="WV", tag="WV")
            WO = const.tile([128, 2048], F16, name="WO", tag="WO")
            BO = const.tile([128, 4], F32, name="BO", tag="BO")
            X = const.tile([128, 4 * NQ], F16, name="X", tag="X")
            CT = const.tile([128, 16 * NQ], F16, name="CT", tag="CT")
            EB = const.tile([128, KC, 2, NQ], F16, name="EB", tag="EB")
            onesF = const.tile([128, 1], F32, name="onesF", tag="onesF")
            nc.vector.memset(onesF, 1.0)

            # ---- loads, ordered by first use across 3 issue queues ----
            # ctx block b holds k-columns [b*512, (b+1)*512) for all 4
            # D-row groups (ki); block 0 is split so the first key chunk
            # (columns 0:128 of each ki group) lands first.
            ctflat = ct_d[:, :]
            ct0a = bass.AP(ctflat.tensor, 0, [[16 * NQ, 128], [512, 4], [1, 128]])
            ct0b = bass.AP(ctflat.tensor, 128, [[16 * NQ, 128], [512, 4], [1, 384]])
            CT0a = bass.AP(CT.tensor, CT.offset, [CT.ap[0], [512, 4], [1, 128]])
            CT0b = bass.AP(CT.tensor, CT.offset + 128, [CT.ap[0], [512, 4], [1, 384]])
            nc.scalar.dma_start(out=CT0a, in_=ct0a)
            nc.scalar.dma_start(out=CT0b, in_=ct0b)
            nc.scalar.dma_start(out=EB[:, 0:4, :, :], in_=eb_d[:, 0:4 * 1024])
            nc.scalar.dma_start(out=WK[:, 512:2048], in_=wk_d[:, 512:2048])
            nc.scalar.dma_start(out=WQ[:, 512:2048], in_=wq_d[:, 512:2048])

            nc.sync.dma_start(out=WK[:, 0:512], in_=wk_d[:, 0:512])
            nc.sync.dma_start(out=WQ[:, 0:512], in_=wq_d[:, 0:512])
            for b in (1, 2, 3):
                csl = slice(b * 2048, (b + 1) * 2048)
                nc.sync.dma_start(out=CT[:, csl], in_=ct_d[:, csl])
            for grp in (1, 2, 3):
                nc.sync.dma_start(
                    out=EB[:, 4 * grp:4 * (grp + 1), :, :],
                    in_=eb_d[:, 4 * grp * 1024:4 * (grp + 1) * 1024])
            nc.sync.dma_start(out=WO, in_=wo_d[:, :])
            nc.sync.dma_start(out=BO, in_=bo_d[:, :])

            nc.gpsimd.dma_start(out=X, in_=x_d[:, :])
            nc.gpsimd.dma_start(out=WV, in_=wv_d[:, :])

            KT = [main.tile([128, NK], F16, name=f"KT{i}", tag=f"KT{i}") for i in range(4)]
            QT = [main.tile([128, NQ], F16, name=f"QT{i}", tag=f"QT{i}") for i in range(4)]
            OT = [main.tile([128, NQ], F16, name=f"OT{i}", tag=f"OT{i}") for i in range(4)]
            Vo = [main.tile([128, HEADS, DH + 1], F16, name=f"Vo{c}", tag=f"Vo{c}")
                  for c in range(KC)]
            for c in range(KC):
                nc.gpsimd.tensor_copy(
                    Vo[c][:, :, DH], onesF[:, 0:1].broadcast_to([128, HEADS]))

            # packed-layout views (weights are mi-major: pair block first)
            def wview(W, ki, mi):
                return W[:, mi * 512 + ki * 128: mi * 512 + (ki + 1) * 128]

            def ctv_cols(b, ki, j0, j1):
                off = b * 2048 + ki * 512
                return CT[:, off + j0: off + j1]

            def ctv_chunk(c, ki):
                return ctv_cols(c // 4, ki, (c % 4) * 128, (c % 4) * 128 + 128)

            def xv(ki):
                return X[:, ki * NQ:(ki + 1) * NQ]

            def wvv(ki):
                return WV[:, ki * 512:(ki + 1) * 512]

            with (
                tc.tile_pool(name="psS", bufs=2, space="PSUM") as psS,
                tc.tile_pool(name="psO", bufs=2, space="PSUM") as psO,
                tc.tile_pool(name="psA", bufs=2, space="PSUM") as psA,
            ):
                def k_proj_cols(mi, nt, j0, j1):
                    ps = psA.tile([128, j1 - j0], F32, name="kproj", tag="proj")
                    for ki in range(4):
                        nc.tensor.matmul(
                            ps, wview(WK, ki, mi), ctv_cols(nt, ki, j0, j1),
                            start=(ki == 0), stop=(ki == 3))
                    nc.vector.tensor_copy(
                        KT[mi][:, nt * 512 + j0: nt * 512 + j1], ps)

                def k_proj_group(mi, nt):
                    k_proj_cols(mi, nt, 0, 512)

                def v_proj_group(c):
                    ps = psA.tile([128, 512], F32, name="vproj", tag="proj")
                    for ki in range(4):
                        nc.tensor.matmul(
                            ps, ctv_chunk(c, ki), wvv(ki),
                            start=(ki == 0), stop=(ki == 3))
                    nc.vector.tensor_copy(
                        Vo[c][:, :, 0:DH],
                        ps.rearrange("p (h d) -> p h d", h=HEADS))

                def q_proj_group(mi):
                    ps = psA.tile([128, 512], F32, name="qproj", tag="proj")
                    for ki in range(4):
                        nc.tensor.matmul(
                            ps, wview(WQ, ki, mi), xv(ki),
                            start=(ki == 0), stop=(ki == 3))
                    nc.vector.tensor_copy(QT[mi], ps)

                def scores_mm(hp, c):
                    lo, hi = slice(0, DH), slice(DH, 128)
                    csl = slice(c * 128, (c + 1) * 128)
                    s = psS.tile([128, 2, NQ], F32, name="s", tag="s")
                    nc.tensor.matmul(
                        s[:, 0, :], KT[hp][lo, csl], QT[hp][lo, :],
                        start=True, stop=True)
                    nc.tensor.matmul(
                        s[:, 1, :], KT[hp][hi, csl], QT[hp][hi, :],
                        start=True, stop=True)
                    return s

                def fillers(hp, c):
                    if hp == 0:
                        if c == 0:
                            k_proj_group(0, 1)
                        if 1 <= c <= 14:
                            v_proj_group(c + 1)
                        if c == 3:
                            k_proj_group(0, 2)
                        elif c == 7:
                            k_proj_group(0, 3)
                        if c == 11:
                            k_proj_group(1, 0)
                        elif c == 12:
                            q_proj_group(1)
                    elif hp == 1:
                        if c in (0, 2, 4):
                            k_proj_group(1, c // 2 + 1)
                        if c in (1, 3, 5, 7):
                            k_proj_group(2, (c - 1) // 2)
                        elif c == 9:
                            q_proj_group(2)
                    elif hp == 2:
                        if c in (1, 3, 5, 7):
                            k_proj_group(3, (c - 1) // 2)
                        elif c == 9:
                            q_proj_group(3)

                # ---- upfront: the minimum for the first two scores ----
                k_proj_cols(0, 0, 0, 128)
                q_proj_group(0)
                s_pipe = [scores_mm(0, 0)]
                k_proj_cols(0, 0, 128, 512)
                s_pipe.append(scores_mm(0, 1))
                v_proj_group(0)
                v_proj_group(1)

                # ---- attention; scores run two chunks ahead ----
                seq = [(hp, c) for hp in range(4) for c in range(KC)]
                for g, (hp, c) in enumerate(seq):
                    h0, h1 = 2 * hp, 2 * hp + 1
                    if c == 0:
                        o2a = psO.tile([DH + 1, NQ], F32, name="o2a", tag="o2")
                        o2b = psO.tile([DH + 1, NQ], F32, name="o2b", tag="o2")
                    s_cur = s_pipe.pop(0)
                    e1 = work.tile([128, 2, NQ], F16, name="e1", tag="e1")
                    nc.scalar.activation(e1, s_cur, AF.Exp)
                    et = work.tile([128, 2, NQ], F16, name="et", tag="et")
                    nc.vector.tensor_mul(et, e1, EB[:, c, :, :])
                    if g + 2 < len(seq):
                        s_pipe.append(scores_mm(*seq[g + 2]))
                    fillers(hp, c)
                    nc.tensor.matmul(
                        o2a, Vo[c][:, h0, :], et[:, 0, :],
                        start=(c == 0), stop=(c == KC - 1))
                    nc.tensor.matmul(
                        o2b, Vo[c][:, h1, :], et[:, 1, :],
                        start=(c == 0), stop=(c == KC - 1))
                    if c != KC - 1:
                        continue
                    # normalization. Lazy DMA path (cheap [128, 4]
                    # reciprocal + DRAM-bounce broadcast) for pairs 0-2;
                    # the last pair stays on-chip (1-lane reciprocal +
                    # PE rank-1 broadcast): no DMA latency in the tail.
                    for h, o2 in ((h0, o2a), (h1, o2b)):
                        rsl = slice((h % 2) * DH, (h % 2) * DH + DH)
                        dq = nc.sync if (hp < 3 or h % 2 == 0) else nc.scalar
                        ss = norm.tile([1, NQ], F32, name="ss", tag="ss")
                        nc.vector.tensor_copy(ss, o2[DH:DH + 1, :])
                        st = norm.tile([128, 4], F32, name="st", tag="st")
                        dq.dma_start(out=st, in_=ss)
                        sr = norm.tile([128, 4], F16, name="sr", tag="sr")
                        nc.vector.reciprocal(sr, st)
                        dq.dma_start(out=scr_d[h][:], in_=sr)
                        cbs = norm.tile([DH, NQ], F16, name="cbs", tag="cbs")
                        dq.dma_start(
                            out=cbs,
                            in_=bass.AP(scr_d[h][:].tensor, 0,
                                        [[0, DH], [1, NQ]]))
                        nc.vector.tensor_mul(OT[hp][rsl, :], o2[0:DH, :], cbs)

            # ---- output projection: all 8 PSUM banks free now; the
            # ki<3 partials overlap the last pair's normalization ----
            with tc.tile_pool(name="psY", bufs=1, space="PSUM") as psY:
                pss = [psY.tile([128, NQ], F32, name=f"yTp{mi}", tag=f"yTp{mi}")
                       for mi in range(4)]
                for ki in range(4):
                    for mi in range(4):
                        nc.tensor.matmul(
                            pss[mi], wview(WO, ki, mi), OT[ki],
                            start=(ki == 0), stop=(ki == 3))
                for mi in range(4):
                    msl = slice(mi * 128, (mi + 1) * 128)
                    yout = norm.tile([128, NQ], F32, name="yout", tag="yout")
                    nc.vector.tensor_scalar_add(yout, pss[mi], BO[:, mi:mi + 1])
                    nc.sync.dma_start(out=yT_d[msl, :], in_=yout)

    nc.compile()
    return nc


_NC_CACHE = {}


def _get_nc():
    if "nc" not in _NC_CACHE:
        _NC_CACHE["nc"] = _build_nc()
    return _NC_CACHE["nc"]


def _pack_w(wT):
    """[512, 512] (ki*128+p, mi*128+j) -> [128, 2048] (p, mi, ki, j)."""
    return np.ascontiguousarray(
        wT.reshape(4, 128, 4, 128).transpose(1, 2, 0, 3).reshape(128, 2048))


def make_in_maps(x, context, bias, Wq, Wk, Wv, Wo, bo):
    x = np.asarray(x, dtype=np.float32)
    context = np.asarray(context, dtype=np.float32)
    bias = np.asarray(bias, dtype=np.float32)
    wq_p = _pack_w((np.asarray(Wq) * SCALE).T.astype(np.float16))
    wk_p = _pack_w(np.asarray(Wk).T.astype(np.float16))
    wo_p = _pack_w(np.asarray(Wo).T.astype(np.float16))
    wvT = np.asarray(Wv).T.astype(np.float16)  # [512 D, 512 (h d)]
    wv_p = np.ascontiguousarray(
        wvT.reshape(4, 128, 512).transpose(1, 0, 2).reshape(128, 2048))
    bo_p = np.ascontiguousarray(
        np.asarray(bo, dtype=np.float32).reshape(4, 128).T)

    in_maps = []
    for core in range(8):
        b, half = core // 2, core % 2
        qs = half * NQ
        xT = x[b, qs:qs + NQ, :].T.astype(np.float16)          # [512 D, 512 q]
        x_p = np.ascontiguousarray(
            xT.reshape(4, 128, NQ).transpose(1, 0, 2).reshape(128, 4 * NQ))
        ctxT = context[b].T.astype(np.float16)                 # [512 D, 2048 k]
        ct_p = np.ascontiguousarray(
            ctxT.reshape(4, 128, 4, 512).transpose(1, 2, 0, 3).reshape(128, 16 * NQ))
        ebT = np.exp(bias[b, qs:qs + NQ, :] - BSHIFT).T.astype(np.float16)
        eb_p = np.ascontiguousarray(
            np.broadcast_to(
                ebT.reshape(KC, 128, 1, NQ).transpose(1, 0, 2, 3),
                (128, KC, 2, NQ)).reshape(128, KC * 2 * NQ))
        in_maps.append({
            "xp": x_p, "ctp": ct_p, "ebp": eb_p,
            "wqp": wq_p, "wkp": wk_p, "wvp": wv_p, "wop": wo_p, "bop": bo_p,
        })
    return in_maps


def kernel(x, context, bias, Wq, Wk, Wv, Wo, bo):
    nc = _get_nc()
    in_maps = make_in_maps(x, context, bias, Wq, Wk, Wv, Wo, bo)
    res = bass_utils.run_bass_kernel_spmd(
        nc, in_maps, core_ids=list(range(8)), trace=False)

    out = np.empty((4, 2 * NQ, D), dtype=np.float32)
    for core in range(8):
        b, half = core // 2, core % 2
        qs = half * NQ
        out[b, qs:qs + NQ, :] = res.results[core]["yT"].T
    return out


# revision 19
# speedup vs baseline: 1.0182x; 1.0182x over previous
"""Cross-attention kernel for Trainium2 (8 NeuronCores, SPMD).

Problem: B=4, Nq=1024, Nk=2048, D=512, 8 heads x 64 head-dim, fp32,
full-tensor bias added to scores before softmax.

Sharding: (batch, query-half) -> 8 disjoint shards, one per core. Each core
computes its own (512, 512) slice of the output; no collectives needed.
K/V projections are computed redundantly on the two cores sharing a batch.

Device layout: attention tensors kept transposed (feature/key dim on
partitions) so every matmul contraction lands on the partition axis:
  QT[d, q] = (SCALE*Wq) @ xT          KT[d, k] = Wk @ ctxT
  V[k, i]  = ctxT.T @ Wv.T
  ST[k, q] = KT_h.T @ QT_h            (two heads of a pair run concurrently
                                       in PE row groups 0-1 / 2-3)
  E = exp(ST) * exp(biasT - 4)        (ACT exp -> elementwise multiply;
                                       host sends exp(bias - 4).T so the
                                       bias add becomes a multiply; the -4
                                       cancels in the normalization)
  out2T[i(+1), q] = [V_h | 1].T @ E   (ones column yields softmax row-sums
                                       in the same accumulation)
  OT = out2T[0:64] * recip(sum)
  yT[d, q] = Wo @ OT + bo

Scheduling (the ACT exp chain, 64 x [128,1024] tiles ~ 71us, is the
pacer; everything else hides under it):
 - scores run TWO chunks ahead of the AV matmuls on the in-order PE
   queue, so neither the exp nor the eB multiply latency gates the PE;
 - eB is stored duplicated ([128, 2, 512] per chunk, duplicated on the
   host) so the multiply is one fully-contiguous fp16 op in the DVE 2x
   mode (slicing or broadcast APs drop it to 1x; GPSIMD is ~3x slower
   per element and its ~1us semaphore waits make fine-grained offload a
   net loss, so all multiplies stay on the DVE);
 - K/Q/V projections for later head pairs ride as PE gap filler;
 - normalization: row sums reshape to [128, 4] by DMA for a cheap
   4-elem/lane reciprocal, then a DRAM-bounce partition broadcast; lazy
   and off the critical path (the last pair's chains run on two DMA
   queues in parallel);
 - the output projection runs entirely in the tail with all 8 PSUM banks
   (ki-outer): the ki<3 partials overlap the last pair's normalization;
 - host packs every input into SBUF-tile layout so each tensor loads with
   few large contiguous DMAs, ordered so the pair-0 K/Q projections
   start after ~1MB of DMA; eB streams per-chunk behind them.
Matmul operands are fp16 (fp32 PSUM accumulate).
"""

import numpy as np
import concourse.bass as bass
import concourse.bacc as bacc
import concourse.mybir as mybir
import concourse.tile as tile
from concourse import bass_utils

HEADS = 8
DH = 64
D = 512
NQ = 512          # queries per core (Nq=1024 split in halves)
NK = 2048
KC = NK // 128    # 16 key chunks
SCALE = DH ** -0.5
BSHIFT = 4.0      # exp(bias - BSHIFT): keeps fp16 weights in range

F32 = mybir.dt.float32
F16 = mybir.dt.float16
AF = mybir.ActivationFunctionType


def _bcast2(ap, n):
    """[128, F] -> [128, n, F] with a step-0 middle dim."""
    return bass.AP(ap.tensor, ap.offset, [ap.ap[0], [0, n], ap.ap[1]])


def _build_nc():
    nc = bacc.Bacc("TRN2", target_bir_lowering=False, debug=False)

    # All inputs host-packed into SBUF-tile layout [128, free].
    x_d = nc.dram_tensor("xp", [128, 4 * NQ], F16, kind="ExternalInput")
    ct_d = nc.dram_tensor("ctp", [128, 16 * NQ], F16, kind="ExternalInput")
    eb_d = nc.dram_tensor("ebp", [128, KC * 2 * NQ], F16, kind="ExternalInput")
    wq_d = nc.dram_tensor("wqp", [128, 2048], F16, kind="ExternalInput")
    wk_d = nc.dram_tensor("wkp", [128, 2048], F16, kind="ExternalInput")
    wv_d = nc.dram_tensor("wvp", [128, 2048], F16, kind="ExternalInput")
    wo_d = nc.dram_tensor("wop", [128, 2048], F16, kind="ExternalInput")
    bo_d = nc.dram_tensor("bop", [128, 4], F32, kind="ExternalInput")
    yT_d = nc.dram_tensor("yT", [D, NQ], F32, kind="ExternalOutput")
    scr_d = [nc.dram_tensor(f"scr{h}", [NQ], F16) for h in range(HEADS)]

    with tile.TileContext(nc) as tc, nc.allow_low_precision(
            reason="fp16 matmul operands, fp32 accumulation"):
        with (
            tc.tile_pool(name="const", bufs=1) as const,
            tc.tile_pool(name="main", bufs=1) as main,
            tc.tile_pool(name="work", bufs=8) as work,
            tc.tile_pool(name="norm", bufs=3) as norm,
        ):
            # ---- persistent SBUF tiles ----
            WK = const.tile([128, 2048], F16, name="WK", tag="WK")
            WQ = const.tile([128, 2048], F16, name="WQ", tag="WQ")
            WV = const.tile([128, 2048], F16, name="WV", tag="WV")
            WO = const.tile([128, 2048], F16, name="WO", tag="WO")
            BO = const.tile([128, 4], F32, name="BO", tag="BO")
            X = const.tile([128, 4 * NQ], F16, name="X", tag="X")
            CT = const.tile([128, 16 * NQ], F16, name="CT", tag="CT")
            EB = const.tile([128, KC, 2, NQ], F16, name="EB", tag="EB")
            onesF = const.tile([128, 1], F32, name="onesF", tag="onesF")
            nc.vector.memset(onesF, 1.0)

            # ---- loads, ordered by first use across 3 issue queues ----
            # ctx block b holds k-columns [b*512, (b+1)*512) for all 4
            # D-row groups (ki); block 0 is split so the first key chunk
            # (columns 0:128 of each ki group) lands first.
            ctflat = ct_d[:, :]
            ct0a = bass.AP(ctflat.tensor, 0, [[16 * NQ, 128], [512, 4], [1, 128]])
            ct0b = bass.AP(ctflat.tensor, 128, [[16 * NQ, 128], [512, 4], [1, 384]])
            CT0a = bass.AP(CT.tensor, CT.offset, [CT.ap[0], [512, 4], [1, 128]])
            CT0b = bass.AP(CT.tensor, CT.offset + 128, [CT.ap[0], [512, 4], [1, 384]])
            nc.scalar.dma_start(out=CT0a, in_=ct0a)
            nc.scalar.dma_start(out=CT0b, in_=ct0b)
            nc.scalar.dma_start(out=EB[:, 0:4, :, :], in_=eb_d[:, 0:4 * 1024])

            nc.sync.dma_start(out=WK[:, 0:512], in_=wk_d[:, 0:512])
            nc.sync.dma_start(out=WQ[:, 0:512], in_=wq_d[:, 0:512])
            for b in (1, 2, 3):
                csl = slice(b * 2048, (b + 1) * 2048)
                nc.sync.dma_start(out=CT[:, csl], in_=ct_d[:, csl])
            for grp in (1, 2, 3):
                nc.sync.dma_start(
                    out=EB[:, 4 * grp:4 * (grp + 1), :, :],
                    in_=eb_d[:, 4 * grp * 1024:4 * (grp + 1) * 1024])
            nc.sync.dma_start(out=WK[:, 512:2048], in_=wk_d[:, 512:2048])
            nc.sync.dma_start(out=WQ[:, 512:2048], in_=wq_d[:, 512:2048])
            nc.sync.dma_start(out=WO, in_=wo_d[:, :])
            nc.sync.dma_start(out=BO, in_=bo_d[:, :])

            nc.gpsimd.dma_start(out=X, in_=x_d[:, :])
            nc.gpsimd.dma_start(out=WV, in_=wv_d[:, :])

            KT = [main.tile([128, NK], F16, name=f"KT{i}", tag=f"KT{i}") for i in range(4)]
            QT = [main.tile([128, NQ], F16, name=f"QT{i}", tag=f"QT{i}") for i in range(4)]
            OT = [main.tile([128, NQ], F16, name=f"OT{i}", tag=f"OT{i}") for i in range(4)]
            Vo = [main.tile([128, HEADS, DH + 1], F16, name=f"Vo{c}", tag=f"Vo{c}")
                  for c in range(KC)]
            for c in range(KC):
                nc.gpsimd.tensor_copy(
                    Vo[c][:, :, DH], onesF[:, 0:1].broadcast_to([128, HEADS]))

            # packed-layout views (weights are mi-major: pair block first)
            def wview(W, ki, mi):
                return W[:, mi * 512 + ki * 128: mi * 512 + (ki + 1) * 128]

            def ctv_cols(b, ki, j0, j1):
                off = b * 2048 + ki * 512
                return CT[:, off + j0: off + j1]

            def ctv_chunk(c, ki):
                return ctv_cols(c // 4, ki, (c % 4) * 128, (c % 4) * 128 + 128)

            def xv(ki):
                return X[:, ki * NQ:(ki + 1) * NQ]

            def wvv(ki):
                return WV[:, ki * 512:(ki + 1) * 512]

            with (
                tc.tile_pool(name="psS", bufs=2, space="PSUM") as psS,
                tc.tile_pool(name="psO", bufs=2, space="PSUM") as psO,
                tc.tile_pool(name="psA", bufs=2, space="PSUM") as psA,
            ):
                def k_proj_cols(mi, nt, j0, j1):
                    ps = psA.tile([128, j1 - j0], F32, name="kproj", tag="proj")
                    for ki in range(4):
                        nc.tensor.matmul(
                            ps, wview(WK, ki, mi), ctv_cols(nt, ki, j0, j1),
                            start=(ki == 0), stop=(ki == 3))
                    nc.vector.tensor_copy(
                        KT[mi][:, nt * 512 + j0: nt * 512 + j1], ps)

                def k_proj_group(mi, nt):
                    k_proj_cols(mi, nt, 0, 512)

                def v_proj_group(c):
                    ps = psA.tile([128, 512], F32, name="vproj", tag="proj")
                    for ki in range(4):
                        nc.tensor.matmul(
                            ps, ctv_chunk(c, ki), wvv(ki),
                            start=(ki == 0), stop=(ki == 3))
                    nc.vector.tensor_copy(
                        Vo[c][:, :, 0:DH],
                        ps.rearrange("p (h d) -> p h d", h=HEADS))

                def q_proj_group(mi):
                    ps = psA.tile([128, 512], F32, name="qproj", tag="proj")
                    for ki in range(4):
                        nc.tensor.matmul(
                            ps, wview(WQ, ki, mi), xv(ki),
                            start=(ki == 0), stop=(ki == 3))
                    nc.vector.tensor_copy(QT[mi], ps)

                def scores_mm(hp, c):
                    lo, hi = slice(0, DH), slice(DH, 128)
                    csl = slice(c * 128, (c + 1) * 128)
                    s = psS.tile([128, 2, NQ], F32, name="s", tag="s")
                    nc.tensor.matmul(
                        s[:, 0, :], KT[hp][lo, csl], QT[hp][lo, :],
                        start=True, stop=True)
                    nc.tensor.matmul(
                        s[:, 1, :], KT[hp][hi, csl], QT[hp][hi, :],
                        start=True, stop=True)
                    return s

                def fillers(hp, c):
                    if hp == 0:
                        if c == 0:
                            k_proj_group(0, 1)
                        if 1 <= c <= 14:
                            v_proj_group(c + 1)
                        if c == 3:
                            k_proj_group(0, 2)
                        elif c == 7:
                            k_proj_group(0, 3)
                        if c == 11:
                            k_proj_group(1, 0)
                        elif c == 12:
                            q_proj_group(1)
                    elif hp == 1:
                        if c in (0, 2, 4):
                            k_proj_group(1, c // 2 + 1)
                        if c in (1, 3, 5, 7):
                            k_proj_group(2, (c - 1) // 2)
                        elif c == 9:
                            q_proj_group(2)
                    elif hp == 2:
                        if c in (1, 3, 5, 7):
                            k_proj_group(3, (c - 1) // 2)
                        elif c == 9:
                            q_proj_group(3)

                # ---- upfront: the minimum for the first two scores ----
                q_proj_group(0)
                k_proj_cols(0, 0, 0, 128)
                s_pipe = [scores_mm(0, 0)]
                k_proj_cols(0, 0, 128, 512)
                s_pipe.append(scores_mm(0, 1))
                v_proj_group(0)
                v_proj_group(1)

                # ---- attention; scores run two chunks ahead ----
                seq = [(hp, c) for hp in range(4) for c in range(KC)]
                for g, (hp, c) in enumerate(seq):
                    h0, h1 = 2 * hp, 2 * hp + 1
                    if c == 0:
                        o2a = psO.tile([DH + 1, NQ], F32, name="o2a", tag="o2")
                        o2b = psO.tile([DH + 1, NQ], F32, name="o2b", tag="o2")
                    s_cur = s_pipe.pop(0)
                    e1 = work.tile([128, 2, NQ], F16, name="e1", tag="e1")
                    nc.scalar.activation(e1, s_cur, AF.Exp)
                    et = work.tile([128, 2, NQ], F16, name="et", tag="et")
                    nc.vector.tensor_mul(et, e1, EB[:, c, :, :])
                    if g + 2 < len(seq):
                        s_pipe.append(scores_mm(*seq[g + 2]))
                    fillers(hp, c)
                    nc.tensor.matmul(
                        o2a, Vo[c][:, h0, :], et[:, 0, :],
                        start=(c == 0), stop=(c == KC - 1))
                    nc.tensor.matmul(
                        o2b, Vo[c][:, h1, :], et[:, 1, :],
                        start=(c == 0), stop=(c == KC - 1))
                    if c != KC - 1:
                        continue
                    # normalization. Lazy DMA path (cheap [128, 4]
                    # reciprocal + DRAM-bounce broadcast) for pairs 0-2;
                    # the last pair stays on-chip (1-lane reciprocal +
                    # PE rank-1 broadcast): no DMA latency in the tail.
                    for h, o2 in ((h0, o2a), (h1, o2b)):
                        rsl = slice((h % 2) * DH, (h % 2) * DH + DH)
                        dq = nc.sync if (hp < 3 or h % 2 == 0) else nc.scalar
                        ss = norm.tile([1, NQ], F32, name="ss", tag="ss")
                        nc.vector.tensor_copy(ss, o2[DH:DH + 1, :])
                        st = norm.tile([128, 4], F32, name="st", tag="st")
                        dq.dma_start(out=st, in_=ss)
                        sr = norm.tile([128, 4], F16, name="sr", tag="sr")
                        nc.vector.reciprocal(sr, st)
                        dq.dma_start(out=scr_d[h][:], in_=sr)
                        cbs = norm.tile([DH, NQ], F16, name="cbs", tag="cbs")
                        dq.dma_start(
                            out=cbs,
                            in_=bass.AP(scr_d[h][:].tensor, 0,
                                        [[0, DH], [1, NQ]]))
                        nc.vector.tensor_mul(OT[hp][rsl, :], o2[0:DH, :], cbs)

            # ---- output projection: all 8 PSUM banks free now; the
            # ki<3 partials overlap the last pair's normalization ----
            with tc.tile_pool(name="psY", bufs=1, space="PSUM") as psY:
                pss = [psY.tile([128, NQ], F32, name=f"yTp{mi}", tag=f"yTp{mi}")
                       for mi in range(4)]
                for ki in range(3):
                    for mi in range(4):
                        nc.tensor.matmul(
                            pss[mi], wview(WO, ki, mi), OT[ki],
                            start=(ki == 0), stop=False)
                for mi in range(4):
                    msl = slice(mi * 128, (mi + 1) * 128)
                    nc.tensor.matmul(
                        pss[mi], wview(WO, 3, mi), OT[3],
                        start=False, stop=True)
                    yout = norm.tile([128, NQ], F32, name="yout", tag="yout")
                    nc.vector.tensor_scalar_add(yout, pss[mi], BO[:, mi:mi + 1])
                    nc.sync.dma_start(out=yT_d[msl, :], in_=yout)

    nc.compile()
    return nc


_NC_CACHE = {}


def _get_nc():
    if "nc" not in _NC_CACHE:
        _NC_CACHE["nc"] = _build_nc()
    return _NC_CACHE["nc"]


def _pack_w(wT):
    """[512, 512] (ki*128+p, mi*128+j) -> [128, 2048] (p, mi, ki, j)."""
    return np.ascontiguousarray(
        wT.reshape(4, 128, 4, 128).transpose(1, 2, 0, 3).reshape(128, 2048))


def make_in_maps(x, context, bias, Wq, Wk, Wv, Wo, bo):
    x = np.asarray(x, dtype=np.float32)
    context = np.asarray(context, dtype=np.float32)
    bias = np.asarray(bias, dtype=np.float32)
    wq_p = _pack_w((np.asarray(Wq) * SCALE).T.astype(np.float16))
    wk_p = _pack_w(np.asarray(Wk).T.astype(np.float16))
    wo_p = _pack_w(np.asarray(Wo).T.astype(np.float16))
    wvT = np.asarray(Wv).T.astype(np.float16)  # [512 D, 512 (h d)]
    wv_p = np.ascontiguousarray(
        wvT.reshape(4, 128, 512).transpose(1, 0, 2).reshape(128, 2048))
    bo_p = np.ascontiguousarray(
        np.asarray(bo, dtype=np.float32).reshape(4, 128).T)

    in_maps = []
    for core in range(8):
        b, half = core // 2, core % 2
        qs = half * NQ
        xT = x[b, qs:qs + NQ, :].T.astype(np.float16)          # [512 D, 512 q]
        x_p = np.ascontiguousarray(
            xT.reshape(4, 128, NQ).transpose(1, 0, 2).reshape(128, 4 * NQ))
        ctxT = context[b].T.astype(np.float16)                 # [512 D, 2048 k]
        ct_p = np.ascontiguousarray(
            ctxT.reshape(4, 128, 4, 512).transpose(1, 2, 0, 3).reshape(128, 16 * NQ))
        ebT = np.exp(bias[b, qs:qs + NQ, :] - BSHIFT).T.astype(np.float16)
        eb_p = np.ascontiguousarray(
            np.broadcast_to(
                ebT.reshape(KC, 128, 1, NQ).transpose(1, 0, 2, 3),
                (128, KC, 2, NQ)).reshape(128, KC * 2 * NQ))
        in_maps.append({
            "xp": x_p, "ctp": ct_p, "ebp": eb_p,
            "wqp": wq_p, "wkp": wk_p, "wvp": wv_p, "wop": wo_p, "bop": bo_p,
        })
    return in_maps


def kernel(x, context, bias, Wq, Wk, Wv, Wo, bo):
    nc = _get_nc()
    in_maps = make_in_maps(x, context, bias, Wq, Wk, Wv, Wo, bo)
    res = bass_utils.run_bass_kernel_spmd(
        nc, in_maps, core_ids=list(range(8)), trace=False)

    out = np.empty((4, 2 * NQ, D), dtype=np.float32)
    for core in range(8):
        b, half = core // 2, core % 2
        qs = half * NQ
        out[b, qs:qs + NQ, :] = res.results[core]["yT"].T
    return out


# revision 20
# speedup vs baseline: 1.0361x; 1.0175x over previous
"""Cross-attention kernel for Trainium2 (8 NeuronCores, SPMD).

Problem: B=4, Nq=1024, Nk=2048, D=512, 8 heads x 64 head-dim, fp32,
full-tensor bias added to scores before softmax.

Sharding: (batch, query-half) -> 8 disjoint shards, one per core. Each core
computes its own (512, 512) slice of the output; no collectives needed.
K/V projections are computed redundantly on the two cores sharing a batch.

Device layout: attention tensors kept transposed (feature/key dim on
partitions) so every matmul contraction lands on the partition axis:
  QT[d, q] = (SCALE*Wq) @ xT          KT[d, k] = Wk @ ctxT
  V[k, i]  = ctxT.T @ Wv.T
  ST[k, q] = KT_h.T @ QT_h            (two heads of a pair run concurrently
                                       in PE row groups 0-1 / 2-3)
  E = exp(ST) * exp(biasT - 4)        (ACT exp -> elementwise multiply;
                                       host sends exp(bias - 4).T so the
                                       bias add becomes a multiply; the -4
                                       cancels in the normalization)
  out2T[i(+1), q] = [V_h | 1].T @ E   (ones column yields softmax row-sums
                                       in the same accumulation)
  OT = out2T[0:64] * recip(sum)
  yT[d, q] = Wo @ OT + bo

Scheduling (the ACT exp chain, 64 x [128,1024] tiles ~ 71us, is the
pacer; everything else hides under it):
 - scores run TWO chunks ahead of the AV matmuls on the in-order PE
   queue, so neither the exp nor the eB multiply latency gates the PE;
 - eB is stored duplicated ([128, 2, 512] per chunk, duplicated on the
   host) so the multiply is one fully-contiguous fp16 op in the DVE 2x
   mode (slicing or broadcast APs drop it to 1x; GPSIMD is ~3x slower
   per element and its ~1us semaphore waits make fine-grained offload a
   net loss, so all multiplies stay on the DVE);
 - K/Q/V projections for later head pairs ride as PE gap filler;
 - normalization: row sums reshape to [128, 4] by DMA for a cheap
   4-elem/lane reciprocal, then a DRAM-bounce partition broadcast; lazy
   and off the critical path (the last pair's chains run on two DMA
   queues in parallel);
 - the output projection runs entirely in the tail with all 8 PSUM banks
   (ki-outer): the ki<3 partials overlap the last pair's normalization;
 - host packs every input into SBUF-tile layout so each tensor loads with
   few large contiguous DMAs, ordered so the pair-0 K/Q projections
   start after ~1MB of DMA; eB streams per-chunk behind them.
Matmul operands are fp16 (fp32 PSUM accumulate).
"""

import numpy as np
import concourse.bass as bass
import concourse.bacc as bacc
import concourse.mybir as mybir
import concourse.tile as tile
from concourse import bass_utils

HEADS = 8
DH = 64
D = 512
NQ = 512          # queries per core (Nq=1024 split in halves)
NK = 2048
KC = NK // 128    # 16 key chunks
SCALE = DH ** -0.5
BSHIFT = 4.0      # exp(bias - BSHIFT): keeps fp16 weights in range

F32 = mybir.dt.float32
F16 = mybir.dt.float16
AF = mybir.ActivationFunctionType


def _bcast2(ap, n):
    """[128, F] -> [128, n, F] with a step-0 middle dim."""
    return bass.AP(ap.tensor, ap.offset, [ap.ap[0], [0, n], ap.ap[1]])


def _build_nc():
    nc = bacc.Bacc("TRN2", target_bir_lowering=False, debug=False)

    # All inputs host-packed into SBUF-tile layout [128, free].
    x_d = nc.dram_tensor("xp", [128, 4 * NQ], F16, kind="ExternalInput")
    ct_d = nc.dram_tensor("ctp", [128, 16 * NQ], F16, kind="ExternalInput")
    eb_d = nc.dram_tensor("ebp", [128, KC * 2 * NQ], F16, kind="ExternalInput")
    wq_d = nc.dram_tensor("wqp", [128, 2048], F16, kind="ExternalInput")
    wk_d = nc.dram_tensor("wkp", [128, 2048], F16, kind="ExternalInput")
    wv_d = nc.dram_tensor("wvp", [128, 2048], F16, kind="ExternalInput")
    wo_d = nc.dram_tensor("wop", [128, 2048], F16, kind="ExternalInput")
    bo_d = nc.dram_tensor("bop", [128, 4], F32, kind="ExternalInput")
    yT_d = nc.dram_tensor("yT", [D, NQ], F32, kind="ExternalOutput")
    scr_d = [nc.dram_tensor(f"scr{h}", [NQ], F16) for h in range(HEADS)]

    with tile.TileContext(nc) as tc, nc.allow_low_precision(
            reason="fp16 matmul operands, fp32 accumulation"):
        with (
            tc.tile_pool(name="const", bufs=1) as const,
            tc.tile_pool(name="main", bufs=1) as main,
            tc.tile_pool(name="work", bufs=8) as work,
            tc.tile_pool(name="norm", bufs=3) as norm,
        ):
            # ---- persistent SBUF tiles ----
            WK = const.tile([128, 2048], F16, name="WK", tag="WK")
            WQ = const.tile([128, 2048], F16, name="WQ", tag="WQ")
            WV = const.tile([128, 2048], F16, name="WV", tag="WV")
            WO = const.tile([128, 2048], F16, name="WO", tag="WO")
            BO = const.tile([128, 4], F32, name="BO", tag="BO")
            X = const.tile([128, 4 * NQ], F16, name="X", tag="X")
            CT = const.tile([128, 16 * NQ], F16, name="CT", tag="CT")
            EB = const.tile([128, KC, 2, NQ], F16, name="EB", tag="EB")
            onesF = const.tile([128, 1], F32, name="onesF", tag="onesF")
            nc.vector.memset(onesF, 1.0)

            # ---- loads, ordered by first use across 3 issue queues ----
            # ctx block b holds k-columns [b*512, (b+1)*512) for all 4
            # D-row groups (ki); block 0 is split so the first key chunk
            # (columns 0:128 of each ki group) lands first.
            ctflat = ct_d[:, :]
            ct0a = bass.AP(ctflat.tensor, 0, [[16 * NQ, 128], [512, 4], [1, 128]])
            ct0b = bass.AP(ctflat.tensor, 128, [[16 * NQ, 128], [512, 4], [1, 384]])
            CT0a = bass.AP(CT.tensor, CT.offset, [CT.ap[0], [512, 4], [1, 128]])
            CT0b = bass.AP(CT.tensor, CT.offset + 128, [CT.ap[0], [512, 4], [1, 384]])
            nc.scalar.dma_start(out=CT0a, in_=ct0a)
            nc.scalar.dma_start(out=CT0b, in_=ct0b)
            nc.scalar.dma_start(out=EB[:, 0:4, :, :], in_=eb_d[:, 0:4 * 1024])

            nc.sync.dma_start(out=WK[:, 0:512], in_=wk_d[:, 0:512])
            nc.sync.dma_start(out=WQ[:, 0:512], in_=wq_d[:, 0:512])
            for b in (1, 2, 3):
                csl = slice(b * 2048, (b + 1) * 2048)
                nc.sync.dma_start(out=CT[:, csl], in_=ct_d[:, csl])
            for grp in (1, 2, 3):
                nc.sync.dma_start(
                    out=EB[:, 4 * grp:4 * (grp + 1), :, :],
                    in_=eb_d[:, 4 * grp * 1024:4 * (grp + 1) * 1024])
            nc.sync.dma_start(out=WK[:, 512:2048], in_=wk_d[:, 512:2048])
            nc.sync.dma_start(out=WQ[:, 512:2048], in_=wq_d[:, 512:2048])
            nc.sync.dma_start(out=WO, in_=wo_d[:, :])
            nc.sync.dma_start(out=BO, in_=bo_d[:, :])

            nc.gpsimd.dma_start(out=X, in_=x_d[:, :])
            nc.gpsimd.dma_start(out=WV, in_=wv_d[:, :])

            KT = [main.tile([128, NK], F16, name=f"KT{i}", tag=f"KT{i}") for i in range(4)]
            QT = [main.tile([128, NQ], F16, name=f"QT{i}", tag=f"QT{i}") for i in range(4)]
            OT = [main.tile([128, NQ], F16, name=f"OT{i}", tag=f"OT{i}") for i in range(4)]
            Vo = [main.tile([128, HEADS, DH + 1], F16, name=f"Vo{c}", tag=f"Vo{c}")
                  for c in range(KC)]
            for c in range(KC):
                nc.gpsimd.tensor_copy(
                    Vo[c][:, :, DH], onesF[:, 0:1].broadcast_to([128, HEADS]))

            # packed-layout views (weights are mi-major: pair block first)
            def wview(W, ki, mi):
                return W[:, mi * 512 + ki * 128: mi * 512 + (ki + 1) * 128]

            def ctv_cols(b, ki, j0, j1):
                off = b * 2048 + ki * 512
                return CT[:, off + j0: off + j1]

            def ctv_chunk(c, ki):
                return ctv_cols(c // 4, ki, (c % 4) * 128, (c % 4) * 128 + 128)

            def xv(ki):
                return X[:, ki * NQ:(ki + 1) * NQ]

            def wvv(ki):
                return WV[:, ki * 512:(ki + 1) * 512]

            with (
                tc.tile_pool(name="psS", bufs=2, space="PSUM") as psS,
                tc.tile_pool(name="psO", bufs=2, space="PSUM") as psO,
                tc.tile_pool(name="psA", bufs=2, space="PSUM") as psA,
            ):
                def k_proj_cols(mi, nt, j0, j1):
                    ps = psA.tile([128, j1 - j0], F32, name="kproj", tag="proj")
                    for ki in range(4):
                        nc.tensor.matmul(
                            ps, wview(WK, ki, mi), ctv_cols(nt, ki, j0, j1),
                            start=(ki == 0), stop=(ki == 3))
                    nc.vector.tensor_copy(
                        KT[mi][:, nt * 512 + j0: nt * 512 + j1], ps)

                def k_proj_group(mi, nt):
                    k_proj_cols(mi, nt, 0, 512)

                def v_proj_group(c):
                    ps = psA.tile([128, 512], F32, name="vproj", tag="proj")
                    for ki in range(4):
                        nc.tensor.matmul(
                            ps, ctv_chunk(c, ki), wvv(ki),
                            start=(ki == 0), stop=(ki == 3))
                    nc.vector.tensor_copy(
                        Vo[c][:, :, 0:DH],
                        ps.rearrange("p (h d) -> p h d", h=HEADS))

                def q_proj_group(mi):
                    ps = psA.tile([128, 512], F32, name="qproj", tag="proj")
                    for ki in range(4):
                        nc.tensor.matmul(
                            ps, wview(WQ, ki, mi), xv(ki),
                            start=(ki == 0), stop=(ki == 3))
                    nc.vector.tensor_copy(QT[mi], ps)

                def scores_mm(hp, c):
                    lo, hi = slice(0, DH), slice(DH, 128)
                    csl = slice(c * 128, (c + 1) * 128)
                    s = psS.tile([128, 2, NQ], F32, name="s", tag="s")
                    nc.tensor.matmul(
                        s[:, 0, :], KT[hp][lo, csl], QT[hp][lo, :],
                        start=True, stop=True)
                    nc.tensor.matmul(
                        s[:, 1, :], KT[hp][hi, csl], QT[hp][hi, :],
                        start=True, stop=True)
                    return s

                def fillers(hp, c):
                    if hp == 0:
                        if c == 1:
                            k_proj_group(0, 3)
                        if 3 <= c <= 14:
                            v_proj_group(c + 1)
                        if c == 11:
                            k_proj_group(1, 0)
                        elif c == 12:
                            q_proj_group(1)
                    elif hp == 1:
                        if c in (0, 2, 4):
                            k_proj_group(1, c // 2 + 1)
                        if c in (1, 3, 5, 7):
                            k_proj_group(2, (c - 1) // 2)
                        elif c == 9:
                            q_proj_group(2)
                    elif hp == 2:
                        if c in (1, 3, 5, 7):
                            k_proj_group(3, (c - 1) // 2)
                        elif c == 9:
                            q_proj_group(3)

                # ---- upfront: scores first, then projections that can
                # run in the PE window while the first exps execute ----
                q_proj_group(0)
                k_proj_cols(0, 0, 0, 128)
                s_pipe = [scores_mm(0, 0)]
                k_proj_cols(0, 0, 128, 512)
                s_pipe.append(scores_mm(0, 1))
                for c in range(4):
                    v_proj_group(c)
                k_proj_group(0, 1)
                k_proj_group(0, 2)

                # ---- attention; scores run two chunks ahead ----
                seq = [(hp, c) for hp in range(4) for c in range(KC)]
                for g, (hp, c) in enumerate(seq):
                    h0, h1 = 2 * hp, 2 * hp + 1
                    if c == 0:
                        o2a = psO.tile([DH + 1, NQ], F32, name="o2a", tag="o2")
                        o2b = psO.tile([DH + 1, NQ], F32, name="o2b", tag="o2")
                    s_cur = s_pipe.pop(0)
                    e1 = work.tile([128, 2, NQ], F16, name="e1", tag="e1")
                    nc.scalar.activation(e1, s_cur, AF.Exp)
                    et = work.tile([128, 2, NQ], F16, name="et", tag="et")
                    nc.vector.tensor_mul(et, e1, EB[:, c, :, :])
                    if g + 2 < len(seq):
                        s_pipe.append(scores_mm(*seq[g + 2]))
                    fillers(hp, c)
                    nc.tensor.matmul(
                        o2a, Vo[c][:, h0, :], et[:, 0, :],
                        start=(c == 0), stop=(c == KC - 1))
                    nc.tensor.matmul(
                        o2b, Vo[c][:, h1, :], et[:, 1, :],
                        start=(c == 0), stop=(c == KC - 1))
                    if c != KC - 1:
                        continue
                    # normalization. Lazy DMA path (cheap [128, 4]
                    # reciprocal + DRAM-bounce broadcast) for pairs 0-2;
                    # the last pair stays on-chip (1-lane reciprocal +
                    # PE rank-1 broadcast): no DMA latency in the tail.
                    for h, o2 in ((h0, o2a), (h1, o2b)):
                        rsl = slice((h % 2) * DH, (h % 2) * DH + DH)
                        dq = nc.sync if (hp < 3 or h % 2 == 0) else nc.scalar
                        ss = norm.tile([1, NQ], F32, name="ss", tag="ss")
                        nc.vector.tensor_copy(ss, o2[DH:DH + 1, :])
                        st = norm.tile([128, 4], F32, name="st", tag="st")
                        dq.dma_start(out=st, in_=ss)
                        sr = norm.tile([128, 4], F16, name="sr", tag="sr")
                        nc.vector.reciprocal(sr, st)
                        dq.dma_start(out=scr_d[h][:], in_=sr)
                        cbs = norm.tile([DH, NQ], F16, name="cbs", tag="cbs")
                        dq.dma_start(
                            out=cbs,
                            in_=bass.AP(scr_d[h][:].tensor, 0,
                                        [[0, DH], [1, NQ]]))
                        nc.vector.tensor_mul(OT[hp][rsl, :], o2[0:DH, :], cbs)

            # ---- output projection: all 8 PSUM banks free now; the
            # ki<3 partials overlap the last pair's normalization ----
            with tc.tile_pool(name="psY", bufs=1, space="PSUM") as psY:
                pss = [psY.tile([128, NQ], F32, name=f"yTp{mi}", tag=f"yTp{mi}")
                       for mi in range(4)]
                for ki in range(3):
                    for mi in range(4):
                        nc.tensor.matmul(
                            pss[mi], wview(WO, ki, mi), OT[ki],
                            start=(ki == 0), stop=False)
                for mi in range(4):
                    msl = slice(mi * 128, (mi + 1) * 128)
                    nc.tensor.matmul(
                        pss[mi], wview(WO, 3, mi), OT[3],
                        start=False, stop=True)
                    yout = norm.tile([128, NQ], F32, name="yout", tag="yout")
                    nc.vector.tensor_scalar_add(yout, pss[mi], BO[:, mi:mi + 1])
                    nc.sync.dma_start(out=yT_d[msl, :], in_=yout)

    nc.compile()
    return nc


_NC_CACHE = {}


def _get_nc():
    if "nc" not in _NC_CACHE:
        _NC_CACHE["nc"] = _build_nc()
    return _NC_CACHE["nc"]


def _pack_w(wT):
    """[512, 512] (ki*128+p, mi*128+j) -> [128, 2048] (p, mi, ki, j)."""
    return np.ascontiguousarray(
        wT.reshape(4, 128, 4, 128).transpose(1, 2, 0, 3).reshape(128, 2048))


def make_in_maps(x, context, bias, Wq, Wk, Wv, Wo, bo):
    x = np.asarray(x, dtype=np.float32)
    context = np.asarray(context, dtype=np.float32)
    bias = np.asarray(bias, dtype=np.float32)
    wq_p = _pack_w((np.asarray(Wq) * SCALE).T.astype(np.float16))
    wk_p = _pack_w(np.asarray(Wk).T.astype(np.float16))
    wo_p = _pack_w(np.asarray(Wo).T.astype(np.float16))
    wvT = np.asarray(Wv).T.astype(np.float16)  # [512 D, 512 (h d)]
    wv_p = np.ascontiguousarray(
        wvT.reshape(4, 128, 512).transpose(1, 0, 2).reshape(128, 2048))
    bo_p = np.ascontiguousarray(
        np.asarray(bo, dtype=np.float32).reshape(4, 128).T)

    in_maps = []
    for core in range(8):
        b, half = core // 2, core % 2
        qs = half * NQ
        xT = x[b, qs:qs + NQ, :].T.astype(np.float16)          # [512 D, 512 q]
        x_p = np.ascontiguousarray(
            xT.reshape(4, 128, NQ).transpose(1, 0, 2).reshape(128, 4 * NQ))
        ctxT = context[b].T.astype(np.float16)                 # [512 D, 2048 k]
        ct_p = np.ascontiguousarray(
            ctxT.reshape(4, 128, 4, 512).transpose(1, 2, 0, 3).reshape(128, 16 * NQ))
        ebT = np.exp(bias[b, qs:qs + NQ, :] - BSHIFT).T.astype(np.float16)
        eb_p = np.ascontiguousarray(
            np.broadcast_to(
                ebT.reshape(KC, 128, 1, NQ).transpose(1, 0, 2, 3),
                (128, KC, 2, NQ)).reshape(128, KC * 2 * NQ))
        in_maps.append({
            "xp": x_p, "ctp": ct_p, "ebp": eb_p,
            "wqp": wq_p, "wkp": wk_p, "wvp": wv_p, "wop": wo_p, "bop": bo_p,
        })
    return in_maps


def kernel(x, context, bias, Wq, Wk, Wv, Wo, bo):
    nc = _get_nc()
    in_maps = make_in_maps(x, context, bias, Wq, Wk, Wv, Wo, bo)
    res = bass_utils.run_bass_kernel_spmd(
        nc, in_maps, core_ids=list(range(8)), trace=False)

    out = np.empty((4, 2 * NQ, D), dtype=np.float32)
    for core in range(8):
        b, half = core // 2, core % 2
        qs = half * NQ
        out[b, qs:qs + NQ, :] = res.results[core]["yT"].T
    return out
